# revision 10
# baseline (speedup 1.0000x reference)
"""Trainium2 Bass kernel for a 2-layer GCN encoder (GCNConv -> tanh -> GCNConv
-> tanh -> linear head), distributed over 8 NeuronCores.

Sharding: nodes are split into 8 contiguous shards (12500 nodes/core, padded
to 12544 = 98 tiles of 128). Edges live on the core that owns their dst node.
Per layer, each core computes g = (x @ W) * dis for its own shard, the g
tables are AllGather'ed (the halo exchange -- the graph is random so the halo
is everything), and each core aggregates messages for its dst tiles with one
indirect-DMA row gather per tile plus selection-matrix matmuls accumulating
in PSUM.

Math (PyG GCNConv with self loops and symmetric normalization):
    deg[v]  = 1 + indegree(v);  dis = deg^-1/2
    g       = (x @ W) * dis[:, None]
    s[v]    = sum_{(u,v) in E} g[u] + g[v]
    out[v]  = tanh(dis[v] * s[v] + b)
The self-loop term g[v] is folded in as one extra gather chunk per tile, and
the bias as a rank-1 matmul (outer(1/dis, b)) into the same PSUM accumulator,
so the epilogue is a single scalar-engine activation: tanh(psum * dis).
"""

import sys

for _p in ("/opt/trn_rl_repo",):
    if _p not in sys.path:
        sys.path.insert(0, _p)

import numpy as np

import concourse.bass as bass
import concourse.bacc as bacc
import concourse.mybir as mybir
from concourse.bass import ts
from concourse.masks import make_identity
from concourse.tile import TileContext

P = 128          # partition width / tile size
N_CORES = 8

# Full-size problem constants (must match reference.setup_inputs)
N_NODES = 100000
N_EDGES = 1600000
IN_DIM = 256
HID = 128
OUT_DIM = 128
N_TYPES = 8


# --------------------------------------------------------------------------
# Host-side graph preprocessing (pure index manipulation, vectorized numpy)
# --------------------------------------------------------------------------

def host_prep(x, edge_index, n_cores=N_CORES):
    """Build per-core input maps. Returns (in_maps_extra, meta) where
    in_maps_extra[c] holds xT/disP/invdis/srcg/dstloc for core c."""
    n = x.shape[0]
    assert n % n_cores == 0
    sh = n // n_cores                      # real nodes per core
    t_per_core = -(-sh // P)               # tiles per core
    shp = t_per_core * P                   # padded shard size

    src = np.asarray(edge_index[0], dtype=np.int64)
    dst = np.asarray(edge_index[1], dtype=np.int64)
    e = src.shape[0]

    deg = np.bincount(dst, minlength=n).astype(np.float64) + 1.0
    dis = (1.0 / np.sqrt(deg)).astype(np.float32)          # deg >= 1 always

    # global tile id of each edge's dst: core * t_per_core + local_tile
    core_of = dst // sh
    loc = dst - core_of * sh
    gtile = core_of * t_per_core + loc // P
    dstloc_of_edge = (loc % P).astype(np.float32)

    order = np.argsort(gtile, kind="stable")
    gtile_s = gtile[order]
    src_s = src[order]
    dstloc_s = dstloc_of_edge[order]

    n_tiles = n_cores * t_per_core
    cnts = np.bincount(gtile_s, minlength=n_tiles)
    starts = np.zeros(n_tiles + 1, dtype=np.int64)
    np.cumsum(cnts, out=starts[1:])
    c_data = int(-(-cnts.max() // P))      # data chunks per tile (uniform)
    ctot = c_data + 1                      # + self-loop chunk

    # gather index into the AllGather'ed table layout:
    # node u lives at row (u // sh) * shp + (u % sh)
    gidx_s = (src_s // sh) * shp + (src_s % sh)

    pos = np.arange(e, dtype=np.int64) - starts[gtile_s]
    chunk = pos // P
    part = pos % P

    # [core, tile, p, chunk] layouts
    srcg = np.zeros((n_cores, t_per_core, P, ctot), dtype=np.int32)
    dstl = np.full((n_cores, t_per_core, P, ctot), -1.0, dtype=np.float32)
    tcore = gtile_s // t_per_core
    ttile = gtile_s % t_per_core
    srcg[tcore, ttile, part, chunk] = gidx_s.astype(np.int32)
    dstl[tcore, ttile, part, chunk] = dstloc_s

    # self-loop chunk (last): partition p of tile t is node t*P+p (if real)
    t_ids = np.arange(t_per_core)[:, None]
    p_ids = np.arange(P)[None, :]
    local_node = t_ids * P + p_ids                          # [T, P]
    valid = local_node < sh
    for c in range(n_cores):
        g_self = np.where(valid, c * shp + local_node, 0)
        srcg[c, :, :, c_data] = g_self.astype(np.int32)
        dstl[c, :, :, c_data] = np.where(valid, p_ids, -1.0).astype(np.float32)

    # SBUF layouts: [128, T*ctot] with tile-major free dim
    srcg_r = np.ascontiguousarray(srcg.transpose(0, 2, 1, 3)).reshape(
        n_cores, P, t_per_core * ctot)
    dstl_r = np.ascontiguousarray(dstl.transpose(0, 2, 1, 3)).reshape(
        n_cores, P, t_per_core * ctot)

    xT = np.asarray(x, dtype=np.float32).T                  # [IN, n]
    in_dim = xT.shape[0]

    in_maps = []
    for c in range(n_cores):
        xs = np.zeros((in_dim, shp), dtype=np.float32)
        xs[:, :sh] = xT[:, c * sh:(c + 1) * sh]
        d = np.zeros(shp, dtype=np.float32)
        d[:sh] = dis[c * sh:(c + 1) * sh]
        disP = np.ascontiguousarray(d.reshape(t_per_core, P).T)     # [128, T]
        in_maps.append({
            "xT": xs,
            "disP": disP,
            "srcg": srcg_r[c],
            "dstloc": dstl_r[c],
        })
    meta = dict(sh=sh, shp=shp, t_per_core=t_per_core, ctot=ctot)
    return in_maps, meta


# --------------------------------------------------------------------------
# Device program
# --------------------------------------------------------------------------

def build_program(t_per_core, ctot, shp, in_dim, hid, out_dim, n_types,
                  n_cores=N_CORES):
    f32 = mybir.dt.float32
    i32 = mybir.dt.int32
    nfull = shp * n_cores
    core_ids = list(range(n_cores))
    T = t_per_core
    assert in_dim % P == 0
    kc_in = in_dim // P
    assert hid == P and out_dim == P

    nc = bacc.Bacc()

    xT = nc.dram_tensor("xT", [in_dim, shp], f32, kind="ExternalInput")
    W1 = nc.dram_tensor("W1", [in_dim, hid], f32, kind="ExternalInput")
    W2 = nc.dram_tensor("W2", [hid, out_dim], f32, kind="ExternalInput")
    W3 = nc.dram_tensor("W3", [out_dim, n_types], f32, kind="ExternalInput")
    B1 = nc.dram_tensor("B1", [P, hid], f32, kind="ExternalInput")
    B2 = nc.dram_tensor("B2", [P, out_dim], f32, kind="ExternalInput")
    B3 = nc.dram_tensor("B3", [P, n_types], f32, kind="ExternalInput")
    disP = nc.dram_tensor("disP", [P, T], f32, kind="ExternalInput")
    srcg = nc.dram_tensor("srcg", [P, T * ctot], i32, kind="ExternalInput")
    dstloc = nc.dram_tensor("dstloc", [P, T * ctot], f32, kind="ExternalInput")

    h2_out = nc.dram_tensor("h2", [shp, out_dim], f32, kind="ExternalOutput")
    lg_out = nc.dram_tensor("logits", [shp, n_types], f32,
                            kind="ExternalOutput")

    g1c = nc.dram_tensor("g1c", [shp, hid], f32)
    g2c = nc.dram_tensor("g2c", [shp, out_dim], f32)
    g1f = nc.dram_tensor("g1f", [nfull, hid], f32, addr_space="Shared")
    g2f = nc.dram_tensor("g2f", [nfull, out_dim], f32, addr_space="Shared")

    mul = mybir.AluOpType.mult
    iseq = mybir.AluOpType.is_equal

    with TileContext(nc) as tc:
        with (
            tc.tile_pool(name="const", bufs=1) as cpool,
            tc.tile_pool(name="work", bufs=3) as wpool,
            tc.tile_pool(name="msgs", bufs=3) as mpool,
            tc.tile_pool(name="sel", bufs=3) as spool,
            tc.tile_pool(name="psum", bufs=2, space="PSUM") as ppool,
            tc.tile_pool(name="psum1", bufs=1, space="PSUM") as ppool1,
        ):
            # ---- constants ----
            ident = cpool.tile([P, P], f32)
            make_identity(nc, ident[:])

            w1t = []
            for k in range(kc_in):
                w = cpool.tile([P, hid], f32, tag=f"w1_{k}")
                nc.sync.dma_start(out=w[:], in_=W1[k * P:(k + 1) * P, :])
                w1t.append(w)
            w2s = cpool.tile([P, out_dim], f32)
            nc.sync.dma_start(out=w2s[:], in_=W2[:, :])
            w3s = cpool.tile([P, n_types], f32)
            nc.sync.dma_start(out=w3s[:], in_=W3[:, :])
            b1s = cpool.tile([P, hid], f32)
            nc.sync.dma_start(out=b1s[:], in_=B1[:, :])
            b2s = cpool.tile([P, out_dim], f32)
            nc.sync.dma_start(out=b2s[:], in_=B2[:, :])
            b3s = cpool.tile([P, n_types], f32)
            nc.sync.dma_start(out=b3s[:], in_=B3[:, :])
            dis_sb = cpool.tile([P, T], f32)
            nc.sync.dma_start(out=dis_sb[:], in_=disP[:, :])
            srcg_sb = cpool.tile([P, T * ctot], i32)
            nc.sync.dma_start(out=srcg_sb[:], in_=srcg[:, :])
            dstl_sb = cpool.tile([P, T * ctot], f32)
            nc.sync.dma_start(out=dstl_sb[:], in_=dstloc[:, :])


            iota_i = cpool.tile([P, ctot * P], i32, tag="iota_i")
            nc.gpsimd.iota(out=iota_i[:], pattern=[[0, ctot], [1, P]],
                           base=0, channel_multiplier=0)
            iota_f = cpool.tile([P, ctot * P], f32)
            nc.vector.tensor_copy(out=iota_f[:], in_=iota_i[:])

            # ---- layer 1 dense: g1 = (x @ W1) * dis ----
            for t in range(T):
                pz = ppool.tile([P, hid], f32, space="PSUM", tag="pz")
                for k in range(kc_in):
                    xt = wpool.tile([P, P], f32, tag=f"xt{k}")
                    nc.sync.dma_start(out=xt[:],
                                      in_=xT[k * P:(k + 1) * P, ts(t, P)])
                    nc.tensor.matmul(out=pz[:], lhsT=xt[:],
                                     rhs=w1t[k][:], start=(k == 0),
                                     stop=(k == kc_in - 1))
                gt = wpool.tile([P, hid], f32, tag="gt")
                nc.vector.tensor_scalar(out=gt[:], in0=pz[:],
                                        scalar1=dis_sb[:, t:t + 1],
                                        scalar2=None, op0=mul)
                nc.sync.dma_start(out=g1c[ts(t, P), :], in_=gt[:])

            # ---- halo exchange 1 ----
            nc.gpsimd.collective_compute(
                "AllGather", mybir.AluOpType.bypass,
                ins=[g1c[:, :]], outs=[g1f[:, :]],
                replica_groups=[core_ids],
            )

            # ---- layer 1 aggregate + layer 2 dense (fused per tile) ----
            for t in range(T):
                msgs = mpool.tile([P, ctot * P], f32, tag="msgs")
                for j in range(ctot):
                    nc.gpsimd.indirect_dma_start(
                        out=msgs[:, ts(j, P)], out_offset=None,
                        in_=g1f[:, :],
                        in_offset=bass.IndirectOffsetOnAxis(
                            ap=srcg_sb[:, t * ctot + j:t * ctot + j + 1],
                            axis=0),
                    )
                sel = spool.tile([P, ctot * P], f32, tag="sel")
                # chunk 0 via a 2-dim tensor_scalar (absorbs the cross-engine
                # waits; the 3-dim TT encoding has too few wait slots)
                nc.vector.tensor_scalar(
                    out=sel[:, 0:P], in0=iota_f[:, 0:P],
                    scalar1=dstl_sb[:, t * ctot:t * ctot + 1],
                    scalar2=None, op0=iseq,
                )
                nc.vector.tensor_tensor(
                    out=sel[:, P:].rearrange("p (c d) -> p c d", d=P),
                    in0=iota_f[:, P:].rearrange("p (c d) -> p c d", d=P),
                    in1=dstl_sb[:, t * ctot + 1:(t + 1) * ctot]
                        .to_broadcast([P, ctot - 1, P]),
                    op=iseq,
                )
                ps = ppool.tile([P, hid], f32, space="PSUM", tag="ps")
                for j in range(ctot):
                    nc.tensor.matmul(out=ps[:], lhsT=sel[:, ts(j, P)],
                                     rhs=msgs[:, ts(j, P)],
                                     start=(j == 0), stop=(j == ctot - 1))
                u1 = wpool.tile([P, hid], f32, tag="u1")
                nc.vector.tensor_scalar(out=u1[:], in0=ps[:],
                                        scalar1=dis_sb[:, t:t + 1],
                                        scalar2=None, op0=mul)
                nc.vector.tensor_tensor(out=u1[:], in0=u1[:], in1=b1s[:],
                                        op=mybir.AluOpType.add)
                h1 = wpool.tile([P, hid], f32, tag="h1")
                nc.scalar.activation(out=h1[:], in_=u1[:],
                                     func=mybir.ActivationFunctionType.Tanh)
                # transpose h1 tile -> lhsT for the layer-2 matmul
                pt = ppool1.tile([P, P], f32, space="PSUM", tag="pt")
                nc.tensor.transpose(out=pt[:], in_=h1[:], identity=ident[:])
                h1t = wpool.tile([P, P], f32, tag="h1t")
                nc.vector.tensor_copy(out=h1t[:], in_=pt[:])
                pz2 = ppool1.tile([P, out_dim], f32, space="PSUM", tag="pz2")
                nc.tensor.matmul(out=pz2[:], lhsT=h1t[:], rhs=w2s[:],
                                 start=True, stop=True)
                g2t = wpool.tile([P, out_dim], f32, tag="g2t")
                nc.vector.tensor_scalar(out=g2t[:], in0=pz2[:],
                                        scalar1=dis_sb[:, t:t + 1],
                                        scalar2=None, op0=mul)
                nc.sync.dma_start(out=g2c[ts(t, P), :], in_=g2t[:])

            # ---- halo exchange 2 ----
            nc.gpsimd.collective_compute(
                "AllGather", mybir.AluOpType.bypass,
                ins=[g2c[:, :]], outs=[g2f[:, :]],
                replica_groups=[core_ids],
            )

            # ---- layer 2 aggregate + head ----
            for t in range(T):
                msgs = mpool.tile([P, ctot * P], f32, tag="msgs")
                for j in range(ctot):
                    nc.gpsimd.indirect_dma_start(
                        out=msgs[:, ts(j, P)], out_offset=None,
                        in_=g2f[:, :],
                        in_offset=bass.IndirectOffsetOnAxis(
                            ap=srcg_sb[:, t * ctot + j:t * ctot + j + 1],
                            axis=0),
                    )
                sel = spool.tile([P, ctot * P], f32, tag="sel")
                # chunk 0 via a 2-dim tensor_scalar (absorbs the cross-engine
                # waits; the 3-dim TT encoding has too few wait slots)
                nc.vector.tensor_scalar(
                    out=sel[:, 0:P], in0=iota_f[:, 0:P],
                    scalar1=dstl_sb[:, t * ctot:t * ctot + 1],
                    scalar2=None, op0=iseq,
                )
                nc.vector.tensor_tensor(
                    out=sel[:, P:].rearrange("p (c d) -> p c d", d=P),
                    in0=iota_f[:, P:].rearrange("p (c d) -> p c d", d=P),
                    in1=dstl_sb[:, t * ctot + 1:(t + 1) * ctot]
                        .to_broadcast([P, ctot - 1, P]),
                    op=iseq,
                )
                ps = ppool.tile([P, out_dim], f32, space="PSUM", tag="ps")
                for j in range(ctot):
                    nc.tensor.matmul(out=ps[:], lhsT=sel[:, ts(j, P)],
                                     rhs=msgs[:, ts(j, P)],
                                     start=(j == 0), stop=(j == ctot - 1))
                u2 = wpool.tile([P, out_dim], f32, tag="u2")
                nc.vector.tensor_scalar(out=u2[:], in0=ps[:],
                                        scalar1=dis_sb[:, t:t + 1],
                                        scalar2=None, op0=mul)
                nc.vector.tensor_tensor(out=u2[:], in0=u2[:], in1=b2s[:],
                                        op=mybir.AluOpType.add)
                h2 = wpool.tile([P, out_dim], f32, tag="h2")
                nc.scalar.activation(out=h2[:], in_=u2[:],
                                     func=mybir.ActivationFunctionType.Tanh)
                nc.sync.dma_start(out=h2_out[ts(t, P), :], in_=h2[:])

                pt = ppool1.tile([P, P], f32, space="PSUM", tag="pt")
                nc.tensor.transpose(out=pt[:], in_=h2[:], identity=ident[:])
                h2t = wpool.tile([P, P], f32, tag="h2t")
                nc.vector.tensor_copy(out=h2t[:], in_=pt[:])
                pl = ppool1.tile([P, n_types], f32, space="PSUM", tag="pl")
                nc.tensor.matmul(out=pl[:], lhsT=h2t[:], rhs=w3s[:],
                                 start=True, stop=True)
                lg = wpool.tile([P, n_types], f32, tag="lg")
                nc.vector.tensor_tensor(out=lg[:], in0=pl[:], in1=b3s[:],
                                        op=mybir.AluOpType.add)
                nc.sync.dma_start(out=lg_out[ts(t, P), :], in_=lg[:])

    return nc


# --------------------------------------------------------------------------
# Entry point
# --------------------------------------------------------------------------

def kernel(x, edge_index, W1, b1, W2, b2, W3, b3, _run=None, _trace=False):
    x = np.asarray(x, dtype=np.float32)
    W1 = np.asarray(W1, dtype=np.float32)
    W2 = np.asarray(W2, dtype=np.float32)
    W3 = np.asarray(W3, dtype=np.float32)
    b1 = np.asarray(b1, dtype=np.float32).reshape(1, -1)
    b2 = np.asarray(b2, dtype=np.float32).reshape(1, -1)
    b3 = np.asarray(b3, dtype=np.float32).reshape(1, -1)

    in_maps, meta = host_prep(x, edge_index)
    sh, shp = meta["sh"], meta["shp"]
    B1 = np.tile(b1.reshape(1, -1), (P, 1)).astype(np.float32)
    B2 = np.tile(b2.reshape(1, -1), (P, 1)).astype(np.float32)
    B3 = np.tile(b3.reshape(1, -1), (P, 1)).astype(np.float32)
    for m in in_maps:
        m.update({"W1": W1, "W2": W2, "W3": W3, "B1": B1, "B2": B2, "B3": B3})

    nc = build_program(meta["t_per_core"], meta["ctot"], shp,
                       x.shape[1], W1.shape[1], W2.shape[1], W3.shape[1])
    nc.finalize()

    from concourse.bass_utils import run_bass_kernel_spmd
    out = run_bass_kernel_spmd(nc, in_maps, list(range(N_CORES)),
                               trace=_trace)
    results = out.results
    h2 = np.concatenate([results[c]["h2"][:sh] for c in range(N_CORES)], 0)
    lg = np.concatenate([results[c]["logits"][:sh] for c in range(N_CORES)], 0)
    if _run is not None:
        _run.append(out)
    return h2, lg


# revision 11
# speedup vs baseline: 1.4487x; 1.4487x over previous
"""Trainium2 Bass kernel for a 2-layer GCN encoder (GCNConv -> tanh -> GCNConv
-> tanh -> linear head), distributed over 8 NeuronCores.

Sharding: nodes are split into 8 contiguous shards (12500 nodes/core, padded
to 12544 = 98 tiles of 128). Edges live on the core that owns their dst node.
Per layer, each core computes g = (x @ W) * dis for its own shard, the g
tables are AllGather'ed (the halo exchange -- the graph is random so the halo
is everything), and each core aggregates messages for its dst tiles with bulk
dma_gather row gathers plus selection-matrix matmuls accumulating in PSUM.

dma_gather uses int16 indices, so the 100352-row gathered table is split into
4 quartile sub-tables of 25088 rows; each core's edges are bucketed by
(dst tile, src quartile) with chunk counts padded uniformly (SPMD: one
program for all cores). The self-loop term is a 5th gather from the core's
own (pre-AllGather) g table.

Math (PyG GCNConv with self loops and symmetric normalization):
    deg[v]  = 1 + indegree(v);  dis = deg^-1/2
    g       = (x @ W) * dis[:, None]
    s[v]    = sum_{(u,v) in E} g[u] + g[v]
    out[v]  = tanh(dis[v] * s[v] + b)
"""

import sys

for _p in ("/opt/trn_rl_repo",):
    if _p not in sys.path:
        sys.path.insert(0, _p)

import numpy as np

import concourse.bass as bass
import concourse.bacc as bacc
import concourse.mybir as mybir
from concourse.bass import ts
from concourse.masks import make_identity
from concourse.tile import TileContext

P = 128          # partition width / tile size
N_CORES = 8
NQ = 4           # quartile sub-tables (int16 index range / 25088 rows)

# Full-size problem constants (must match reference.setup_inputs)
N_NODES = 100000
N_EDGES = 1600000
IN_DIM = 256
HID = 128
OUT_DIM = 128
N_TYPES = 8


def _pack16(v):
    """[..., M] int16 message-index list -> [..., 128, M//16] SBUF layout:
    message m at partition m%16 (replicated across the 8 groups of 16)."""
    m = v.shape[-1]
    assert m % 16 == 0
    w = np.swapaxes(v.reshape(*v.shape[:-1], m // 16, 16), -1, -2)
    return np.broadcast_to(
        w[..., None, :, :], (*v.shape[:-1], 8, 16, m // 16)
    ).reshape(*v.shape[:-1], 128, m // 16).astype(np.int16)


# --------------------------------------------------------------------------
# Host-side graph preprocessing (pure index manipulation, vectorized numpy)
# --------------------------------------------------------------------------

def host_prep(x, edge_index, n_cores=N_CORES):
    n = x.shape[0]
    assert n % n_cores == 0
    sh = n // n_cores                      # real nodes per core
    t_per_core = -(-sh // P)               # tiles per core
    T = t_per_core
    shp = T * P                            # padded shard size
    nfull = shp * n_cores
    assert nfull % NQ == 0
    qrows = nfull // NQ
    assert qrows <= 32768

    src = np.asarray(edge_index[0], dtype=np.int64)
    dst = np.asarray(edge_index[1], dtype=np.int64)
    e = src.shape[0]

    deg = np.bincount(dst, minlength=n).astype(np.float64) + 1.0
    dis = (1.0 / np.sqrt(deg)).astype(np.float32)

    core_of = dst // sh
    loc = dst - core_of * sh
    gtile = core_of * T + loc // P                     # global dst tile
    dstloc_of_edge = (loc % P).astype(np.float32)
    gidx = (src // sh) * shp + (src % sh)              # row in gathered table
    quart = gidx // qrows

    cell = gtile * NQ + quart
    order = np.argsort(cell, kind="stable")
    cell_s = cell[order]
    gidx_s = gidx[order]
    quart_s = quart[order]
    dstloc_s = dstloc_of_edge[order]

    n_cells = n_cores * T * NQ
    cnts = np.bincount(cell_s, minlength=n_cells)
    starts = np.zeros(n_cells + 1, dtype=np.int64)
    np.cumsum(cnts, out=starts[1:])
    CQ = int(-(-cnts.max() // P))          # chunks per (tile, quartile)
    CT = NQ * CQ + 1                       # chunks per tile (+ self)

    G = min(4, T)                          # tiles per gather group
    NG = -(-T // G)
    TP = NG * G                            # padded tile count

    pos = np.arange(e, dtype=np.int64) - starts[cell_s]
    j = pos // P
    p = pos % P

    tcore = cell_s // (T * NQ)
    ttile = (cell_s // NQ) % T

    A = np.zeros((n_cores, TP, NQ, CQ, P), dtype=np.int16)
    D = np.full((n_cores, TP, CT, P), -1.0, dtype=np.float32)
    A[tcore, ttile, quart_s, j, p] = (gidx_s - quart_s * qrows).astype(np.int16)
    D[tcore, ttile, quart_s * CQ + j, p] = dstloc_s

    # self chunk (index CT-1): gather own g rows from the local table
    t_ids = np.arange(TP)[:, None]
    p_ids = np.arange(P)[None, :]
    local_node = t_ids * P + p_ids                      # [TP, P]
    valid = local_node < sh
    B = np.where(valid, local_node, 0).astype(np.int16)  # same for every core
    for c in range(n_cores):
        D[c, :, CT - 1, :] = np.where(valid, p_ids, -1.0).astype(np.float32)

    # pack gather indices
    # quartile instruction (group gi, q): messages A[c, gi*G:(gi+1)*G, q]
    Aq = A.reshape(n_cores, NG, G, NQ, CQ, P).transpose(0, 1, 3, 2, 4, 5)
    Aq = np.ascontiguousarray(Aq).reshape(n_cores, NG * NQ, G * CQ * P)
    SQ = (G * CQ * P) // 16
    IDXQ = _pack16(Aq)                                  # [c, NG*NQ, 128, SQ]
    IDXQ = np.ascontiguousarray(IDXQ.transpose(0, 2, 1, 3)).reshape(
        n_cores, P, NG * NQ * SQ)

    Bs = B.reshape(NG, G * P)
    SS = (G * P) // 16
    IDXS = _pack16(Bs)                                  # [NG, 128, SS]
    IDXS = np.ascontiguousarray(IDXS.transpose(1, 0, 2)).reshape(P, NG * SS)

    dstl_r = np.ascontiguousarray(D.transpose(0, 3, 1, 2)).reshape(
        n_cores, P, TP * CT)

    xT = np.asarray(x, dtype=np.float32).T              # [IN, n]
    in_dim = xT.shape[0]

    in_maps = []
    for c in range(n_cores):
        xs = np.zeros((in_dim, shp), dtype=np.float32)
        xs[:, :sh] = xT[:, c * sh:(c + 1) * sh]
        d = np.zeros(shp, dtype=np.float32)
        d[:sh] = dis[c * sh:(c + 1) * sh]
        disP = np.ascontiguousarray(d.reshape(T, P).T)  # [128, T]
        in_maps.append({
            "xT": xs,
            "disP": disP,
            "idxq": IDXQ[c],
            "idxs": IDXS,
            "dstloc": dstl_r[c],
        })
    meta = dict(sh=sh, shp=shp, t_per_core=T, CQ=CQ, CT=CT, G=G, NG=NG,
                qrows=qrows)
    return in_maps, meta


# --------------------------------------------------------------------------
# Device program
# --------------------------------------------------------------------------

def build_program(T, CQ, G, NG, shp, qrows, in_dim, hid, out_dim, n_types,
                  n_cores=N_CORES):
    f32 = mybir.dt.float32
    i16 = mybir.dt.int16
    i32 = mybir.dt.int32
    nfull = shp * n_cores
    core_ids = list(range(n_cores))
    CT = NQ * CQ + 1
    TP = NG * G
    SQ = (G * CQ * P) // 16
    SS = (G * P) // 16
    assert in_dim % P == 0
    kc_in = in_dim // P
    assert hid == P and out_dim == P

    nc = bacc.Bacc()

    xT = nc.dram_tensor("xT", [in_dim, shp], f32, kind="ExternalInput")
    W1 = nc.dram_tensor("W1", [in_dim, hid], f32, kind="ExternalInput")
    W2 = nc.dram_tensor("W2", [hid, out_dim], f32, kind="ExternalInput")
    W3 = nc.dram_tensor("W3", [out_dim, n_types], f32, kind="ExternalInput")
    B1 = nc.dram_tensor("B1", [P, hid], f32, kind="ExternalInput")
    B2 = nc.dram_tensor("B2", [P, out_dim], f32, kind="ExternalInput")
    B3 = nc.dram_tensor("B3", [P, n_types], f32, kind="ExternalInput")
    disP = nc.dram_tensor("disP", [P, T], f32, kind="ExternalInput")
    idxq = nc.dram_tensor("idxq", [P, NG * NQ * SQ], i16, kind="ExternalInput")
    idxs = nc.dram_tensor("idxs", [P, NG * SS], i16, kind="ExternalInput")
    dstloc = nc.dram_tensor("dstloc", [P, TP * CT], f32, kind="ExternalInput")

    h2_out = nc.dram_tensor("h2", [shp, out_dim], f32, kind="ExternalOutput")
    lg_out = nc.dram_tensor("logits", [shp, n_types], f32,
                            kind="ExternalOutput")

    g1c = nc.dram_tensor("g1c", [shp, hid], f32)
    g2c = nc.dram_tensor("g2c", [shp, out_dim], f32)
    g1f = nc.dram_tensor("g1f", [nfull, hid], f32, addr_space="Shared")
    g2f = nc.dram_tensor("g2f", [nfull, out_dim], f32, addr_space="Shared")

    mul = mybir.AluOpType.mult
    add = mybir.AluOpType.add
    iseq = mybir.AluOpType.is_equal

    with TileContext(nc) as tc:
        with (
            tc.tile_pool(name="const", bufs=1) as cpool,
            tc.tile_pool(name="work", bufs=3) as wpool,
            tc.tile_pool(name="msgs", bufs=2) as mpool,
            tc.tile_pool(name="sel", bufs=2) as spool,
            tc.tile_pool(name="psum", bufs=2, space="PSUM") as ppool,
            tc.tile_pool(name="psum1", bufs=1, space="PSUM") as ppool1,
        ):
            # ---- constants ----
            ident = cpool.tile([P, P], f32)
            make_identity(nc, ident[:])

            w1t = []
            for k in range(kc_in):
                w = cpool.tile([P, hid], f32, tag=f"w1_{k}")
                nc.sync.dma_start(out=w[:], in_=W1[k * P:(k + 1) * P, :])
                w1t.append(w)
            w2s = cpool.tile([P, out_dim], f32)
            nc.sync.dma_start(out=w2s[:], in_=W2[:, :])
            w3s = cpool.tile([P, n_types], f32)
            nc.sync.dma_start(out=w3s[:], in_=W3[:, :])
            b1s = cpool.tile([P, hid], f32)
            nc.sync.dma_start(out=b1s[:], in_=B1[:, :])
            b2s = cpool.tile([P, out_dim], f32)
            nc.sync.dma_start(out=b2s[:], in_=B2[:, :])
            b3s = cpool.tile([P, n_types], f32)
            nc.sync.dma_start(out=b3s[:], in_=B3[:, :])
            dis_sb = cpool.tile([P, T], f32)
            nc.sync.dma_start(out=dis_sb[:], in_=disP[:, :])
            idxq_sb = cpool.tile([P, NG * NQ * SQ], i16)
            nc.sync.dma_start(out=idxq_sb[:], in_=idxq[:, :])
            idxs_sb = cpool.tile([P, NG * SS], i16)
            nc.sync.dma_start(out=idxs_sb[:], in_=idxs[:, :])
            dstl_sb = cpool.tile([P, TP * CT], f32)
            nc.sync.dma_start(out=dstl_sb[:], in_=dstloc[:, :])

            iota_i = cpool.tile([P, CT * P], i32, tag="iota_i")
            nc.gpsimd.iota(out=iota_i[:], pattern=[[0, CT], [1, P]],
                           base=0, channel_multiplier=0)
            iota_f = cpool.tile([P, CT * P], f32)
            nc.vector.tensor_copy(out=iota_f[:], in_=iota_i[:])

            # ---- layer 1 dense: g1 = (x @ W1) * dis ----
            for t in range(T):
                pz = ppool.tile([P, hid], f32, space="PSUM", tag="pz")
                for k in range(kc_in):
                    xt = wpool.tile([P, P], f32, tag=f"xt{k}")
                    nc.sync.dma_start(out=xt[:],
                                      in_=xT[k * P:(k + 1) * P, ts(t, P)])
                    nc.tensor.matmul(out=pz[:], lhsT=xt[:],
                                     rhs=w1t[k][:], start=(k == 0),
                                     stop=(k == kc_in - 1))
                gt = wpool.tile([P, hid], f32, tag="gt")
                nc.vector.tensor_scalar(out=gt[:], in0=pz[:],
                                        scalar1=dis_sb[:, t:t + 1],
                                        scalar2=None, op0=mul)
                nc.sync.dma_start(out=g1c[ts(t, P), :], in_=gt[:])

            # ---- halo exchange 1 ----
            nc.gpsimd.collective_compute(
                "AllGather", mybir.AluOpType.bypass,
                ins=[g1c[:, :]], outs=[g1f[:, :]],
                replica_groups=[core_ids],
            )

            def aggregate(gf, gc, epilogue):
                """Grouped bulk gathers + selection matmuls + epilogue."""
                for gi in range(NG):
                    mq = []
                    for q in range(NQ):
                        m = mpool.tile([P, G * CQ * P], f32, tag=f"mq{q}")
                        nc.gpsimd.dma_gather(
                            out_ap=m[:].rearrange("p (c f) -> p c f", f=P),
                            in_ap=gf[q * qrows:(q + 1) * qrows, :],
                            idxs_ap=idxq_sb[:, (gi * NQ + q) * SQ:
                                            (gi * NQ + q + 1) * SQ],
                            num_idxs=G * CQ * P, num_idxs_reg=G * CQ * P,
                            elem_size=P, single_packet=False,
                        )
                        mq.append(m)
                    ms = mpool.tile([P, G * P], f32, tag="ms")
                    nc.gpsimd.dma_gather(
                        out_ap=ms[:].rearrange("p (c f) -> p c f", f=P),
                        in_ap=gc[:, :],
                        idxs_ap=idxs_sb[:, gi * SS:(gi + 1) * SS],
                        num_idxs=G * P, num_idxs_reg=G * P,
                        elem_size=P, single_packet=False,
                    )
                    for tl in range(G):
                        t = gi * G + tl
                        if t >= T:
                            continue
                        c0 = t * CT
                        sel = spool.tile([P, CT * P], f32, tag="sel")
                        # chunk 0 via 2-dim tensor_scalar (wait-slot limits
                        # forbid multi-wait on the 3-dim TT encoding)
                        nc.vector.tensor_scalar(
                            out=sel[:, 0:P], in0=iota_f[:, 0:P],
                            scalar1=dstl_sb[:, c0:c0 + 1],
                            scalar2=None, op0=iseq,
                        )
                        nc.vector.tensor_tensor(
                            out=sel[:, P:].rearrange("p (c d) -> p c d", d=P),
                            in0=iota_f[:, P:].rearrange("p (c d) -> p c d",
                                                        d=P),
                            in1=dstl_sb[:, c0 + 1:c0 + CT]
                                .to_broadcast([P, CT - 1, P]),
                            op=iseq,
                        )
                        ps = ppool.tile([P, P], f32, space="PSUM", tag="ps")
                        for q in range(NQ):
                            for jj in range(CQ):
                                k = q * CQ + jj
                                nc.tensor.matmul(
                                    out=ps[:], lhsT=sel[:, ts(k, P)],
                                    rhs=mq[q][:, ts(tl * CQ + jj, P)],
                                    start=(k == 0), stop=False)
                        nc.tensor.matmul(out=ps[:], lhsT=sel[:, ts(CT - 1, P)],
                                         rhs=ms[:, ts(tl, P)],
                                         start=False, stop=True)
                        epilogue(t, ps)

            # ---- layer 1 aggregate + layer 2 dense (fused per tile) ----
            def epi1(t, ps):
                u1 = wpool.tile([P, hid], f32, tag="u1")
                nc.vector.tensor_scalar(out=u1[:], in0=ps[:],
                                        scalar1=dis_sb[:, t:t + 1],
                                        scalar2=None, op0=mul)
                nc.vector.tensor_tensor(out=u1[:], in0=u1[:], in1=b1s[:],
                                        op=add)
                h1 = wpool.tile([P, hid], f32, tag="h1")
                nc.scalar.activation(out=h1[:], in_=u1[:],
                                     func=mybir.ActivationFunctionType.Tanh)
                pt = ppool1.tile([P, P], f32, space="PSUM", tag="pt")
                nc.tensor.transpose(out=pt[:], in_=h1[:], identity=ident[:])
                h1t = wpool.tile([P, P], f32, tag="h1t")
                nc.vector.tensor_copy(out=h1t[:], in_=pt[:])
                pz2 = ppool1.tile([P, out_dim], f32, space="PSUM", tag="pz2")
                nc.tensor.matmul(out=pz2[:], lhsT=h1t[:], rhs=w2s[:],
                                 start=True, stop=True)
                g2t = wpool.tile([P, out_dim], f32, tag="g2t")
                nc.vector.tensor_scalar(out=g2t[:], in0=pz2[:],
                                        scalar1=dis_sb[:, t:t + 1],
                                        scalar2=None, op0=mul)
                nc.sync.dma_start(out=g2c[ts(t, P), :], in_=g2t[:])

            aggregate(g1f, g1c, epi1)

            # ---- halo exchange 2 ----
            nc.gpsimd.collective_compute(
                "AllGather", mybir.AluOpType.bypass,
                ins=[g2c[:, :]], outs=[g2f[:, :]],
                replica_groups=[core_ids],
            )

            # ---- layer 2 aggregate + head ----
            def epi2(t, ps):
                u2 = wpool.tile([P, out_dim], f32, tag="u2")
                nc.vector.tensor_scalar(out=u2[:], in0=ps[:],
                                        scalar1=dis_sb[:, t:t + 1],
                                        scalar2=None, op0=mul)
                nc.vector.tensor_tensor(out=u2[:], in0=u2[:], in1=b2s[:],
                                        op=add)
                h2 = wpool.tile([P, out_dim], f32, tag="h2")
                nc.scalar.activation(out=h2[:], in_=u2[:],
                                     func=mybir.ActivationFunctionType.Tanh)
                nc.sync.dma_start(out=h2_out[ts(t, P), :], in_=h2[:])
                pt = ppool1.tile([P, P], f32, space="PSUM", tag="pt")
                nc.tensor.transpose(out=pt[:], in_=h2[:], identity=ident[:])
                h2t = wpool.tile([P, P], f32, tag="h2t")
                nc.vector.tensor_copy(out=h2t[:], in_=pt[:])
                pl = ppool1.tile([P, n_types], f32, space="PSUM", tag="pl")
                nc.tensor.matmul(out=pl[:], lhsT=h2t[:], rhs=w3s[:],
                                 start=True, stop=True)
                lg = wpool.tile([P, n_types], f32, tag="lg")
                nc.vector.tensor_tensor(out=lg[:], in0=pl[:], in1=b3s[:],
                                        op=add)
                nc.sync.dma_start(out=lg_out[ts(t, P), :], in_=lg[:])

            aggregate(g2f, g2c, epi2)

    return nc


# --------------------------------------------------------------------------
# Entry point
# --------------------------------------------------------------------------

def kernel(x, edge_index, W1, b1, W2, b2, W3, b3, _run=None, _trace=False):
    x = np.asarray(x, dtype=np.float32)
    W1 = np.asarray(W1, dtype=np.float32)
    W2 = np.asarray(W2, dtype=np.float32)
    W3 = np.asarray(W3, dtype=np.float32)
    b1 = np.asarray(b1, dtype=np.float32).reshape(1, -1)
    b2 = np.asarray(b2, dtype=np.float32).reshape(1, -1)
    b3 = np.asarray(b3, dtype=np.float32).reshape(1, -1)

    in_maps, meta = host_prep(x, edge_index)
    sh, shp = meta["sh"], meta["shp"]
    B1 = np.tile(b1, (P, 1)).astype(np.float32)
    B2 = np.tile(b2, (P, 1)).astype(np.float32)
    B3 = np.tile(b3, (P, 1)).astype(np.float32)
    for m in in_maps:
        m.update({"W1": W1, "W2": W2, "W3": W3, "B1": B1, "B2": B2, "B3": B3})

    nc = build_program(meta["t_per_core"], meta["CQ"], meta["G"], meta["NG"],
                       shp, meta["qrows"],
                       x.shape[1], W1.shape[1], W2.shape[1], W3.shape[1])
    nc.finalize()

    from concourse.bass_utils import run_bass_kernel_spmd
    out = run_bass_kernel_spmd(nc, in_maps, list(range(N_CORES)),
                               trace=_trace)
    results = out.results
    h2 = np.concatenate([results[c]["h2"][:sh] for c in range(N_CORES)], 0)
    lg = np.concatenate([results[c]["logits"][:sh] for c in range(N_CORES)], 0)
    if _run is not None:
        _run.append(out)
    return h2, lg
